# revision 11
# baseline (speedup 1.0000x reference)
"""Trainium2 Bass kernel for nn_FCVI_Net_78864189489850.

Computation (reference):
  L = lower-tri scatter of cov_vector (exp on diag)          [769, 769]
  samples = mean + L @ z                                      [769, S, B]
  W0 = samples[0:256], b0 = samples[256:512],
  W1 = samples[512:768], b1 = samples[768]
  h = relu(x * W0 + b0);  out = sum_o h * W1 + b1             [S, B]

Strategy (8 NeuronCores, batch-sharded, no cross-device comms):
  - Host builds L (cheap scatter + 769 exps), transposes to LT, casts to
    f16.  Each core gets a B-shard of z (columns c = s*256 + b_local,
    4096 cols) in f16 — halves HBM traffic; f16 matmul runs at full rate.
  - Transposed-orientation matmuls: sT[c, i] = sum_k z[k,c] * LT[k,i]
    with triangular k-tile skip.  Per 128-column tile, 10 matmuls:
      pA[:, 0:512]   <- k-tiles 0,1 (N=512; merged i-ranges)
      pA[:, 256:512] <- k-tiles 2,3 (N=256)
      pB[:, 0:257]   <- k-tiles 0..5 (N=257, i in [512, 769))
  - All mean terms ride on DVE constants (a = x*mean0 + mean1 per batch
    parity, m2b broadcast), the b1 row's k=768 term + mean768 comes in
    via a host-precomputed affine z8p[p, m] = L[768,768]*z[768,c] +
    mean[768].
  - Per c-tile: ACT scales sT0 by per-partition x, DVE adds sT1 + a,
    ACT applies relu, DVE multiplies by (sT2 + m2b) and row-reduces via
    scalar_tensor_tensor's accumulator.  Output staged [128, 32];
    host reassembles [16, 2048].
"""
import os
import numpy as np

P = 769
S = 16
B = 2048
NCORES = 8
BC = B // NCORES          # 256 batch per core
NCOL = S * BC             # 4096 columns per core
NCT = NCOL // 128         # 32 c-tiles per core
NCHUNK = 8                # z DMA chunks
CHW = NCOL // NCHUNK      # 512

_cache = {}


def _mm_dtype():
    import concourse.mybir as mybir
    name = os.environ.get("BASS_FCVI_DTYPE", "f16")
    return {
        "f16": (mybir.dt.float16, np.float16),
        "f32r": (mybir.dt.float32r, np.float32),
    }[name]


def _build_program():
    import concourse.bacc as bacc
    import concourse.tile as tile
    from concourse import mybir

    mmdt, _ = _mm_dtype()
    f32 = mybir.dt.float32

    nc = bacc.Bacc("TRN2", target_bir_lowering=False, debug=False)

    za_d = nc.dram_tensor("za", [768, NCOL], mmdt, kind="ExternalInput")
    lt_d = nc.dram_tensor("lt", [768, P], mmdt, kind="ExternalInput")
    cst_d = nc.dram_tensor("cst", [128, 802], f32, kind="ExternalInput")
    out_d = nc.dram_tensor("out", [128, NCT], f32, kind="ExternalOutput")

    # LT column ranges stored per k-tile (LT[k, i] == 0 for i < k)
    LT_COLS = [(0, 769), (0, 769), (256, 769), (256, 769),
               (512, 769), (512, 769)]

    with tile.TileContext(nc) as tc:
        with (
            tc.tile_pool(name="zpool", bufs=1) as zpool,
            tc.tile_pool(name="ltpool", bufs=1) as ltpool,
            tc.tile_pool(name="cpool", bufs=1) as cpool,
            tc.tile_pool(name="work", bufs=4) as work,
            tc.tile_pool(name="gsc", bufs=3) as gsc,
            tc.tile_pool(name="pa", bufs=4, space="PSUM") as pa_pool,
            tc.tile_pool(name="pb", bufs=4, space="PSUM") as pb_pool,
        ):
            # --- DMAs, ordered so c-tile 0's dependencies land first ---
            ltt = [None] * 6
            zc = [None] * NCHUNK

            def load_lt(t):
                lo, hi = LT_COLS[t]
                tl = ltpool.tile([128, hi - lo], mmdt, tag=f"lt{t}")
                nc.sync.dma_start(
                    out=tl[:], in_=lt_d.ap()[t * 128:(t + 1) * 128, lo:hi])
                ltt[t] = tl

            def load_zc(q):
                zq = zpool.tile([128, 6, CHW], mmdt, tag=f"zc{q}")
                src = za_d.ap()[:, q * CHW:(q + 1) * CHW].rearrange(
                    "(t p) c -> p t c", p=128)
                nc.sync.dma_start(out=zq[:], in_=src)
                zc[q] = zq

            zc0h = []
            for h_ in range(2):
                zq = zpool.tile([128, 6, CHW // 2], mmdt, tag=f"zc0{h_}")
                src = za_d.ap()[:, h_ * (CHW // 2):(h_ + 1) * (CHW // 2)].rearrange(
                    "(t p) c -> p t c", p=128)
                nc.sync.dma_start(out=zq[:], in_=src)
                zc0h.append(zq)
            load_lt(0)
            load_lt(1)

            cst = cpool.tile([128, 802], f32, tag="cst")
            nc.sync.dma_start(out=cst[:], in_=cst_d.ap()[:, :])
            m01 = cst[:, 0:512]
            m2b = cst[:, 512:768]
            z8p = cst[:, 768:800]
            xvt = cst[:, 800:802]

            load_lt(2)
            load_lt(3)
            load_zc(1)
            load_lt(4)
            load_lt(5)
            for q in range(2, NCHUNK):
                load_zc(q)

            # a[parity] = x * mean0 + mean1   (two one-time DVE ops)
            apar = []
            for par in range(2):
                a = cpool.tile([128, 256], f32, tag=f"a{par}")
                nc.vector.scalar_tensor_tensor(
                    out=a[:], in0=m01[:, 0:256], scalar=xvt[:, par:par + 1],
                    in1=m01[:, 256:512],
                    op0=mybir.AluOpType.mult, op1=mybir.AluOpType.add)
                apar.append(a)

            stag = cpool.tile([128, NCT], f32, tag="stag")
            s3stag = cpool.tile([128, NCT], f32, tag="s3stag")
            stag2 = cpool.tile([128, NCT], f32, tag="stag2")

            def rhs(t, g0, g1):
                lo, _ = LT_COLS[t]
                return ltt[t][:, g0 - lo:g1 - lo]

            for m in range(NCT):
                q, cl = divmod(m * 128, CHW)

                def lhsT(t):
                    if q == 0:
                        return zc0h[m // 2][:, t, (m % 2) * 128:(m % 2) * 128 + 128]
                    return zc[q][:, t, cl:cl + 128]

                pA = pa_pool.tile([128, 512], f32, tag="pA")
                pB = pb_pool.tile([128, 257], f32, tag="pB")
                # k-tiles 0,1 cover i in [0, 512) in one N=512 matmul each
                nc.tensor.matmul(pA[:, 0:512], lhsT(0), rhs(0, 0, 512),
                                 start=True, stop=False)
                nc.tensor.matmul(pA[:, 0:512], lhsT(1), rhs(1, 0, 512),
                                 start=False, stop=False)
                # k-tiles 2,3 only contribute to i in [256, 512)
                nc.tensor.matmul(pA[:, 256:512], lhsT(2), rhs(2, 256, 512),
                                 start=False, stop=False)
                nc.tensor.matmul(pA[:, 256:512], lhsT(3), rhs(3, 256, 512),
                                 start=False, stop=True)
                # i in [512, 769): k-tiles 0..5
                for t in range(6):
                    nc.tensor.matmul(pB[:, 0:257], lhsT(t), rhs(t, 512, 769),
                                     start=(t == 0), stop=(t == 5))

                # t = x * sT0   (ACT: PSUM->SBUF copy with per-partition scale)
                t_ = work.tile([128, 256], f32, tag="t")
                nc.scalar.activation(t_[:], pA[:, 0:256],
                                     mybir.ActivationFunctionType.Copy,
                                     scale=xvt[:, m % 2:m % 2 + 1])
                # u = t + sT1
                u = work.tile([128, 256], f32, tag="u")
                nc.vector.tensor_add(u[:], t_[:], pA[:, 256:512])
                # u2 = u + (x*mean0 + mean1)
                u2 = work.tile([128, 256], f32, tag="u2")
                nc.vector.tensor_add(u2[:], u[:], apar[m % 2][:])
                # h = relu(u2)
                h = work.tile([128, 256], f32, tag="h")
                nc.scalar.activation(h[:], u2[:],
                                     mybir.ActivationFunctionType.Relu)
                # v = sT2 + mean2
                v = work.tile([128, 256], f32, tag="v")
                nc.vector.tensor_add(v[:], pB[:, 0:256], m2b)
                # g = h * v ; stag[:, m] = sum_o g
                g = gsc.tile([128, 256], f32, tag="g")
                nc.vector.scalar_tensor_tensor(
                    out=g[:], in0=h[:], scalar=1.0, in1=v[:],
                    op0=mybir.AluOpType.mult, op1=mybir.AluOpType.mult,
                    accum_out=stag[:, m:m + 1])
                # b1: s3stag[:, m] = (sum_{k<768} L[768,k] z[k,c]) + z8p
                nc.vector.tensor_add(s3stag[:, m:m + 1], pB[:, 256:257],
                                     z8p[:, m:m + 1])

                if m in (NCT // 2 - 1, NCT - 1):
                    h_ = 0 if m == NCT // 2 - 1 else 1
                    sl = slice(h_ * (NCT // 2), (h_ + 1) * (NCT // 2))
                    nc.vector.tensor_add(stag2[:, sl], stag[:, sl],
                                         s3stag[:, sl])
                    nc.sync.dma_start(out=out_d.ap()[:, sl], in_=stag2[:, sl])

    nc.compile()
    return nc


def _prep_inputs(x, mean, cov_vector, z):
    _, npdt = _mm_dtype()

    L = np.zeros((P, P), dtype=np.float32)
    L[np.tril_indices(P)] = cov_vector
    d = np.diag(L).copy()
    L[np.diag_indices(P)] = np.exp(d)

    lt = np.ascontiguousarray(L.T[:768]).astype(npdt)     # rows k in [0, 768)

    cst_base = np.empty((128, 802), dtype=np.float32)
    cst_base[:, 0:512] = mean[None, 0:512]
    cst_base[:, 512:768] = mean[None, 512:768]

    z2 = z.reshape(P, S, B)
    in_maps = []
    for c in range(NCORES):
        zs = z2[:, :, c * BC:(c + 1) * BC].reshape(P, NCOL)
        za = zs[:768].astype(npdt)
        # z8p[p, m] = L[768,768] * z[768, 128m + p] + mean[768]
        z8 = zs[768].astype(np.float32)                    # [4096]
        cst = cst_base.copy()
        cst[:, 768:800] = (L[768, 768] * z8 + mean[768]).reshape(NCT, 128).T
        xs = x[c * BC:(c + 1) * BC]
        cst[:, 800] = xs[0:128]
        cst[:, 801] = xs[128:256]
        in_maps.append({"za": np.ascontiguousarray(za), "lt": lt,
                        "cst": cst})
    return in_maps


def _assemble(results):
    out = np.empty((S, B), dtype=np.float32)
    for c in range(NCORES):
        o = results[c]["out"]                       # [128, 32]
        oc = o.reshape(128, S, 2).transpose(1, 2, 0).reshape(S, BC)
        out[:, c * BC:(c + 1) * BC] = oc
    return out


def _run(inputs, trace=False, trace_kwargs=None):
    from concourse.bass_utils import run_bass_kernel_spmd

    key = os.environ.get("BASS_FCVI_DTYPE", "f16")
    if key not in _cache:
        _cache[key] = _build_program()
    nc = _cache[key]

    in_maps = _prep_inputs(**inputs)
    kw = {}
    if trace:
        kw["trace"] = True
        if trace_kwargs:
            kw.update(trace_kwargs)
    res = run_bass_kernel_spmd(nc, in_maps, core_ids=list(range(NCORES)), **kw)
    return _assemble(res.results), res


def kernel(x, mean, cov_vector, z):
    out, _ = _run(dict(x=np.asarray(x), mean=np.asarray(mean),
                       cov_vector=np.asarray(cov_vector), z=np.asarray(z)))
    return out
